# revision 36
# baseline (speedup 1.0000x reference)
"""Trainium2 Bass kernel for nn_Aggregate (gate-softmax graph pooling).

Computes, for each graph b:
    gate[b,n] = x[b,n,:] @ W1 + b1
    attn      = softmax(gate[b,:])
    y[b,:]    = sum_n attn[b,n] * x[b,n,:]

Strategy (memory-bound; roofline = one HBM read of x at ~360 GB/s/core
= ~93 us for the 32 MiB/core shard):
  - Data-parallel over the 32 graphs: 4 graphs per NeuronCore, 8 cores.
  - Single pass over x. gate values are ~N(0,1) so exp() without the
    max-shift is safe in fp32; softmax = (sum e^g x) / (sum e^g) needs
    no running-max correction, so every x element is read from HBM once.
  - Per 1 MiB slab (1024 nodes as [128 partitions x 8 nodes x 256 feat],
    8 KiB contiguous per partition -> full-rate DMA descriptors):
      DVE    : g1 = x * W1 (bf16 out, bf16 W1 operand - both halve SBUF
               write/read bytes; measured ~1.1 ns/col vs 1.8 all-fp32)
               for node-groups 0:5, then one grouped reduce of the bf16
               g1 -> gates fp32 for the same groups.
      GpSimd : g1 for node-groups 5:8 (one flat [P,768] multiply).
      ACT    : Copy+accum_out reduces groups 5:8, then exp(gates+b1) ->
               w bf16 into the graph's weight wall (no per-slab
               accumulator drain: the softmax denominator is ONE
               Copy+accum over the whole wall per graph, saving the
               279ns READ_ACC per slab).
      PE     : per pair of node-groups, matmul(psum[2,512] += w[128,2].T
               @ g1[128,512]), all bf16: 1 cycle/row vs 4 for the fp32
               matmul that bottlenecked the original (155us of PE). g1 =
               x*bf16(W1) as the moving operand yields y[f]*W1[f]; the
               host divides by bf16(W1)[f], which cancels exactly (bf16
               error is relative, so nothing blows up for small W1[f]).
  - Denominator finishes on host: sum of the per-partition exp-sums.
"""

import sys
import types

import numpy as np

import concourse.bass as bass
import concourse.tile as tile
from concourse import mybir
from concourse.bass_utils import run_bass_kernel_spmd

# bass_utils' axon trace path does `from antenv.axon_hooks import ...`, which
# this image doesn't ship; stub it so BASS_TRACE=1 degrades to a warning
# instead of an ImportError. (Our own profiling wires a real hook in test.py.)
if "antenv.axon_hooks" not in sys.modules:
    try:
        import antenv  # noqa: F401
        import antenv.axon_hooks  # noqa: F401
    except ImportError:
        _m = types.ModuleType("antenv.axon_hooks")
        _m._hook = None
        _m.set_axon_ntff_profile_hook = lambda h: setattr(_m, "_hook", h)
        _m.get_axon_ntff_profile_hook = lambda: _m._hook
        sys.modules["antenv.axon_hooks"] = _m

BZ, N, F = 32, 8192, 256
NCORES = 8
BZL = BZ // NCORES  # graphs per core
P = 128             # SBUF partitions
JJ = 8              # node-groups per slab
JD = 5              # node-groups multiplied on DVE
JR = 5              # node-groups reduced on DVE
SLAB = P * JJ       # 1024 nodes per slab
FP32 = mybir.dt.float32
BF16 = mybir.dt.bfloat16


def split_multiwait(nc) -> int:
    """Walrus in this image only encodes one sync-wait per instruction for
    ctrl-class ops; hoist extra waits onto single-wait NoOps just before."""
    n_fixed = 0
    for fn in nc.m.functions:
        for blk in fn.blocks:
            new_list = []
            for inst in blk.instructions:
                si = inst.sync_info
                waits = list(si.on_wait) if si is not None else []
                if len(waits) > 1:
                    for k, w in enumerate(waits):
                        new_list.append(
                            mybir.InstNoOp(
                                name=f"{inst.name}-wsplit{k}",
                                engine=inst.engine,
                                sync_info=mybir.SyncInfo(on_wait=[w], on_update=[]),
                                bass_nofuse=True,
                            )
                        )
                    inst.sync_info = mybir.SyncInfo(
                        on_wait=[], on_update=list(si.on_update)
                    )
                    n_fixed += 1
                new_list.append(inst)
            blk.instructions = new_list
    return n_fixed


def build(n_nodes: int = N, bzl: int = BZL, fixup: bool = True) -> bass.Bass:
    nslab = n_nodes // SLAB
    assert nslab * SLAB == n_nodes

    nc = bass.Bass("TRN2", target_bir_lowering=False, debug=False)
    x_d = nc.dram_tensor("x", [bzl, n_nodes, F], FP32, kind="ExternalInput").ap()
    w1_d = nc.dram_tensor("W1b", [JJ * F], BF16, kind="ExternalInput").ap()
    b1_d = nc.dram_tensor("b1", [1], FP32, kind="ExternalInput").ap()
    y_d = nc.dram_tensor("y_unnorm", [bzl, 2, 2 * F], FP32, kind="ExternalOutput").ap()
    dn_d = nc.dram_tensor("denom", [bzl, P, 1], FP32, kind="ExternalOutput").ap()

    def flat(ap):
        return ap.rearrange("p j f -> p (j f)")

    with tile.TileContext(nc) as tc:
        with (
            tc.tile_pool(name="singles", bufs=1) as singles,
            tc.tile_pool(name="xp", bufs=10) as xp,
            tc.tile_pool(name="g1p", bufs=3) as g1p,
            tc.tile_pool(name="scrp", bufs=4) as scrp,
            tc.tile_pool(name="gatesp", bufs=3) as gatesp,
            tc.tile_pool(name="wallp", bufs=3) as wallp,
            tc.tile_pool(name="dnp", bufs=4) as dnp,
            tc.tile_pool(name="outp", bufs=4) as outp,
            tc.tile_pool(name="psum", bufs=2, space="PSUM") as psump,
        ):
            # b1 scalar broadcast to [128,1] (the exp bias) - first in the
            # sync queue so the first slab's exp is never blocked on it.
            b1b = singles.tile([P, 1], FP32)
            nc.sync.dma_start(
                out=b1b,
                in_=bass.AP(tensor=b1_d.tensor, offset=b1_d.offset, ap=[[0, P], [1, 1]]),
            )
            # W1 arrives pre-converted to bf16 [256]; one DRAM->SBUF
            # broadcast DMA materializes the [128, 8, 256] operand with no
            # engine work and no cast on the critical path.
            w1rep8 = singles.tile([P, JJ, F], BF16)
            nc.sync.dma_start(
                out=w1rep8,
                in_=bass.AP(
                    tensor=w1_d.tensor,
                    offset=w1_d.offset,
                    ap=[[0, P], [1, JJ * F]],
                ),
            )
            # Dummy exp so ACT's table set loads during the preamble instead
            # of on the first real exp.
            warm = singles.tile([P, 1], FP32)
            nc.scalar.activation(
                out=warm, in_=b1b, func=mybir.ActivationFunctionType.Exp,
                bias=0.0, scale=1.0,
            )

            outs = []
            for b in range(bzl):
                # per-graph wall of softmax weights: [P, nslab*JJ] bf16
                wall = wallp.tile([P, nslab * JJ], BF16, tag="wall", name=f"wall_{b}")
                psum_row = psump.tile([2, 2 * F], FP32, tag="psum_row",
                                      name=f"psum_row_{b}")
                # Two consecutive slabs share one g1/gates tile in SLOT
                # order [dve_s0 (5) | dve_s1 (5) | gp_s0 (3) | gp_s1 (3)]:
                # the f-reduce of both slabs' DVE groups is ONE [P,10,256]
                # grouped reduce (big grouped reduces run ~0.88 ns/col vs
                # 1.16 for 5-group ones).
                ND = JJ - JD  # gp groups per slab (3)

                def slot_of(sl, j):
                    return sl * JD + j if j < JD else 2 * JD + ND * sl + (j - JD)

                for sp in range(nslab // 2):
                    g1 = g1p.tile([P, 2 * JJ, F], BF16, tag="g1")
                    gates = gatesp.tile([P, 2 * JJ], FP32, tag="gates")
                    for sl in range(2):
                        s = 2 * sp + sl
                        x_sb = xp.tile([P, JJ, F], FP32, tag="x_sb")
                        nc.sync.dma_start(
                            out=x_sb,
                            in_=x_d[b, s * SLAB : (s + 1) * SLAB, :].rearrange(
                                "(p j) f -> p j f", p=P
                            ),
                        )
                        for j in range(JD, JJ):
                            slot = slot_of(sl, j)
                            nc.gpsimd.tensor_mul(
                                flat(g1[:, slot : slot + 1, :]),
                                flat(x_sb[:, j : j + 1, :]),
                                flat(w1rep8[:, j : j + 1, :]),
                            )
                        nc.vector.tensor_mul(
                            flat(g1[:, sl * JD : (sl + 1) * JD, :]),
                            flat(x_sb[:, 0:JD, :]),
                            flat(w1rep8[:, 0:JD, :]),
                        )
                        for j in range(JD, JJ):
                            slot = slot_of(sl, j)
                            scr = scrp.tile([P, F], BF16, tag="scr")
                            nc.scalar.activation(
                                out=scr,
                                in_=g1[:, slot, :],
                                func=mybir.ActivationFunctionType.Copy,
                                bias=0.0,
                                scale=1.0,
                                accum_out=gates[:, slot : slot + 1],
                            )
                    nc.vector.reduce_sum(
                        gates[:, 0 : 2 * JD], g1[:, 0 : 2 * JD, :],
                        axis=mybir.AxisListType.X,
                    )
                    for sl in range(2):
                        s = 2 * sp + sl
                        # two exps per slab: DVE-produced gate slots and
                        # ACT-produced ones are not contiguous.
                        nc.scalar.activation(
                            out=wall[:, s * JJ : s * JJ + JD],
                            in_=gates[:, sl * JD : (sl + 1) * JD],
                            func=mybir.ActivationFunctionType.Exp,
                            bias=b1b,
                            scale=1.0,
                        )
                        nc.scalar.activation(
                            out=wall[:, s * JJ + JD : (s + 1) * JJ],
                            in_=gates[:, 2 * JD + ND * sl : 2 * JD + ND * (sl + 1)],
                            func=mybir.ActivationFunctionType.Exp,
                            bias=b1b,
                            scale=1.0,
                        )
                        for t in range(JJ // 2):
                            a = slot_of(sl, 2 * t)
                            c = slot_of(sl, 2 * t + 1)
                            base = g1[:, a, :]
                            rhs = bass.AP(
                                tensor=base.tensor,
                                offset=base.offset,
                                ap=[list(base.ap[0]), [(c - a) * F, 2], [1, F]],
                            )
                            nc.tensor.matmul(
                                out=psum_row,
                                lhsT=wall[:, s * JJ + 2 * t : s * JJ + 2 * t + 2],
                                rhs=rhs,
                                start=(s == 0 and t == 0),
                                stop=(s == nslab - 1 and t == JJ // 2 - 1),
                            )
                # Softmax denominator: one fused accumulate over the wall.
                dn = dnp.tile([P, 1], FP32, tag="dn")
                scr = scrp.tile([P, nslab * JJ], BF16, tag="scr3")
                nc.scalar.activation(
                    out=scr,
                    in_=wall,
                    func=mybir.ActivationFunctionType.Copy,
                    bias=0.0,
                    scale=1.0,
                    accum_out=dn,
                )
                yrow = outp.tile([2, 2 * F], FP32, tag="yrow", name=f"yrow_{b}")
                nc.scalar.copy(yrow, psum_row)
                outs.append((b, yrow, dn))
            # Output DMAs issued after every slab DMA is enqueued: the sync
            # queue never blocks the x stream on a graph's last matmul.
            for b, yrow, dn in outs:
                nc.sync.dma_start(out=y_d[b], in_=yrow)
                nc.sync.dma_start(out=dn_d[b], in_=dn)

    if fixup:
        # CoreSim chokes on the inserted NoOps; only needed for the HW compile.
        split_multiwait(nc)
    return nc


def run(x, W1, b1, trace: bool = False, tmpdir: str | None = None):
    """Shard over cores, execute, and return (y, BassKernelResults)."""
    import ml_dtypes

    x = np.ascontiguousarray(np.asarray(x, dtype=np.float32))
    W1 = np.ascontiguousarray(np.asarray(W1, dtype=np.float32))
    b1 = np.ascontiguousarray(np.asarray(b1, dtype=np.float32))
    assert x.shape == (BZ, N, F), x.shape

    nc = build()
    w1b16 = np.ascontiguousarray(np.tile(W1[:, 0].astype(ml_dtypes.bfloat16), JJ))
    in_maps = [
        {"x": np.ascontiguousarray(x[c * BZL : (c + 1) * BZL]), "W1b": w1b16,
         "b1": b1}
        for c in range(NCORES)
    ]
    res = run_bass_kernel_spmd(
        nc, in_maps, core_ids=list(range(NCORES)), trace=trace, tmpdir=tmpdir
    )
    y2 = np.concatenate([r["y_unnorm"] for r in res.results], axis=0)  # [32,2,512]
    y_un = y2[:, 0, 0:F] + y2[:, 1, F : 2 * F]                           # [32, 256]
    w1b = W1[:, 0].astype(ml_dtypes.bfloat16).astype(np.float64)
    y_un = y_un / w1b                   # PE consumed g1 = x*bf16(W1); undo
    dn = np.concatenate([r["denom"] for r in res.results], axis=0)   # [32,128,1]
    denom = dn.reshape(BZ, -1).astype(np.float64).sum(axis=1)
    y = (y_un / denom[:, None]).astype(np.float32)
    return y, res


def kernel(x, W1, b1):
    y, _ = run(x, W1, b1)
    return y


# revision 38
# speedup vs baseline: 1.0062x; 1.0062x over previous
"""Trainium2 Bass kernel for nn_Aggregate (gate-softmax graph pooling).

Computes, for each graph b:
    gate[b,n] = x[b,n,:] @ W1 + b1
    attn      = softmax(gate[b,:])
    y[b,:]    = sum_n attn[b,n] * x[b,n,:]

Strategy (memory-bound; roofline = one HBM read of x at ~360 GB/s/core
= ~93 us for the 32 MiB/core shard):
  - Data-parallel over the 32 graphs: 4 graphs per NeuronCore, 8 cores.
  - Single pass over x. gate values are ~N(0,1) so exp() without the
    max-shift is safe in fp32; softmax = (sum e^g x) / (sum e^g) needs
    no running-max correction, so every x element is read from HBM once.
  - Per 1 MiB slab (1024 nodes as [128 partitions x 8 nodes x 256 feat],
    8 KiB contiguous per partition -> full-rate DMA descriptors):
      DVE    : g1 = x * W1 (bf16 out, bf16 W1 operand - both halve SBUF
               write/read bytes; measured ~1.1 ns/col vs 1.8 all-fp32)
               for node-groups 0:5, then one grouped reduce of the bf16
               g1 -> gates fp32 for the same groups.
      GpSimd : g1 for node-groups 5:8 (one flat [P,768] multiply).
      ACT    : Copy+accum_out reduces groups 5:8, then exp(gates+b1) ->
               w bf16 into the graph's weight wall (no per-slab
               accumulator drain: the softmax denominator is ONE
               Copy+accum over the whole wall per graph, saving the
               279ns READ_ACC per slab).
      PE     : per pair of node-groups, matmul(psum[2,512] += w[128,2].T
               @ g1[128,512]), all bf16: 1 cycle/row vs 4 for the fp32
               matmul that bottlenecked the original (155us of PE). g1 =
               x*bf16(W1) as the moving operand yields y[f]*W1[f]; the
               host divides by bf16(W1)[f], which cancels exactly (bf16
               error is relative, so nothing blows up for small W1[f]).
  - Denominator finishes on host: sum of the per-partition exp-sums.
"""

import sys
import types

import numpy as np

import concourse.bass as bass
import concourse.tile as tile
from concourse import mybir
from concourse.bass_utils import run_bass_kernel_spmd

# bass_utils' axon trace path does `from antenv.axon_hooks import ...`, which
# this image doesn't ship; stub it so BASS_TRACE=1 degrades to a warning
# instead of an ImportError. (Our own profiling wires a real hook in test.py.)
if "antenv.axon_hooks" not in sys.modules:
    try:
        import antenv  # noqa: F401
        import antenv.axon_hooks  # noqa: F401
    except ImportError:
        _m = types.ModuleType("antenv.axon_hooks")
        _m._hook = None
        _m.set_axon_ntff_profile_hook = lambda h: setattr(_m, "_hook", h)
        _m.get_axon_ntff_profile_hook = lambda: _m._hook
        sys.modules["antenv.axon_hooks"] = _m

BZ, N, F = 32, 8192, 256
NCORES = 8
BZL = BZ // NCORES  # graphs per core
P = 128             # SBUF partitions
JJ = 8              # node-groups per slab
JD = 5              # node-groups multiplied on DVE
JR = 5              # node-groups reduced on DVE
SLAB = P * JJ       # 1024 nodes per slab
FP32 = mybir.dt.float32
BF16 = mybir.dt.bfloat16


def split_multiwait(nc) -> int:
    """Walrus in this image only encodes one sync-wait per instruction for
    ctrl-class ops; hoist extra waits onto single-wait NoOps just before."""
    n_fixed = 0
    for fn in nc.m.functions:
        for blk in fn.blocks:
            new_list = []
            for inst in blk.instructions:
                si = inst.sync_info
                waits = list(si.on_wait) if si is not None else []
                if len(waits) > 1:
                    for k, w in enumerate(waits):
                        new_list.append(
                            mybir.InstNoOp(
                                name=f"{inst.name}-wsplit{k}",
                                engine=inst.engine,
                                sync_info=mybir.SyncInfo(on_wait=[w], on_update=[]),
                                bass_nofuse=True,
                            )
                        )
                    inst.sync_info = mybir.SyncInfo(
                        on_wait=[], on_update=list(si.on_update)
                    )
                    n_fixed += 1
                new_list.append(inst)
            blk.instructions = new_list
    return n_fixed


def build(n_nodes: int = N, bzl: int = BZL, fixup: bool = True) -> bass.Bass:
    nslab = n_nodes // SLAB
    assert nslab * SLAB == n_nodes

    nc = bass.Bass("TRN2", target_bir_lowering=False, debug=False)
    x_d = nc.dram_tensor("x", [bzl, n_nodes, F], FP32, kind="ExternalInput").ap()
    w1_d = nc.dram_tensor("W1b", [JJ * F], BF16, kind="ExternalInput").ap()
    b1_d = nc.dram_tensor("b1", [1], FP32, kind="ExternalInput").ap()
    y_d = nc.dram_tensor("y_unnorm", [bzl, 2, 2 * F], FP32, kind="ExternalOutput").ap()
    dn_d = nc.dram_tensor("denom", [bzl, P, 1], FP32, kind="ExternalOutput").ap()

    def flat(ap):
        return ap.rearrange("p j f -> p (j f)")

    with tile.TileContext(nc) as tc:
        with (
            tc.tile_pool(name="singles", bufs=1) as singles,
            tc.tile_pool(name="xp", bufs=10) as xp,
            tc.tile_pool(name="g1p", bufs=3) as g1p,
            tc.tile_pool(name="scrp", bufs=4) as scrp,
            tc.tile_pool(name="gatesp", bufs=3) as gatesp,
            tc.tile_pool(name="wallp", bufs=3) as wallp,
            tc.tile_pool(name="dnp", bufs=4) as dnp,
            tc.tile_pool(name="outp", bufs=4) as outp,
            tc.tile_pool(name="psum", bufs=2, space="PSUM") as psump,
        ):
            # b1 scalar broadcast to [128,1] (the exp bias) - first in the
            # sync queue so the first slab's exp is never blocked on it.
            b1b = singles.tile([P, 1], FP32)
            nc.sync.dma_start(
                out=b1b,
                in_=bass.AP(tensor=b1_d.tensor, offset=b1_d.offset, ap=[[0, P], [1, 1]]),
            )
            # W1 arrives pre-converted to bf16 [256]; one DRAM->SBUF
            # broadcast DMA materializes the [128, 8, 256] operand with no
            # engine work and no cast on the critical path.
            w1rep8 = singles.tile([P, JJ, F], BF16)
            nc.sync.dma_start(
                out=w1rep8,
                in_=bass.AP(
                    tensor=w1_d.tensor,
                    offset=w1_d.offset,
                    ap=[[0, P], [1, JJ * F]],
                ),
            )
            # Dummy exp so ACT's table set loads during the preamble instead
            # of on the first real exp.
            warm = singles.tile([P, 1], FP32)
            nc.scalar.activation(
                out=warm, in_=b1b, func=mybir.ActivationFunctionType.Exp,
                bias=0.0, scale=1.0,
            )

            outs = []
            for b in range(bzl):
                # per-graph wall of softmax weights: [P, nslab*JJ] bf16
                wall = wallp.tile([P, nslab * JJ], BF16, tag="wall", name=f"wall_{b}")
                psum_row = psump.tile([2, 2 * F], FP32, tag="psum_row",
                                      name=f"psum_row_{b}")
                # Two consecutive slabs share one g1/gates tile in SLOT
                # order [dve_s0 (5) | dve_s1 (5) | gp_s0 (3) | gp_s1 (3)]:
                # the f-reduce of both slabs' DVE groups is ONE [P,10,256]
                # grouped reduce (big grouped reduces run ~0.88 ns/col vs
                # 1.16 for 5-group ones).
                ND = JJ - JD  # gp groups per slab (3)

                def slot_of(sl, j):
                    return sl * JD + j if j < JD else 2 * JD + ND * sl + (j - JD)

                pending = [None]

                def flush_pair_tail():
                    if pending[0] is not None:
                        pending[0]()
                        pending[0] = None

                def emit_slab_front(g1, gates, sp, sl):
                    s = 2 * sp + sl
                    x_sb = xp.tile([P, JJ, F], FP32, tag="x_sb")
                    nc.sync.dma_start(
                        out=x_sb,
                        in_=x_d[b, s * SLAB : (s + 1) * SLAB, :].rearrange(
                            "(p j) f -> p j f", p=P
                        ),
                    )
                    for j in range(JD, JJ):
                        slot = slot_of(sl, j)
                        nc.gpsimd.tensor_mul(
                            flat(g1[:, slot : slot + 1, :]),
                            flat(x_sb[:, j : j + 1, :]),
                            flat(w1rep8[:, j : j + 1, :]),
                        )
                    nc.vector.tensor_mul(
                        flat(g1[:, sl * JD : (sl + 1) * JD, :]),
                        flat(x_sb[:, 0:JD, :]),
                        flat(w1rep8[:, 0:JD, :]),
                    )
                    for j in range(JD, JJ):
                        slot = slot_of(sl, j)
                        scr = scrp.tile([P, F], BF16, tag="scr")
                        nc.scalar.activation(
                            out=scr,
                            in_=g1[:, slot, :],
                            func=mybir.ActivationFunctionType.Copy,
                            bias=0.0,
                            scale=1.0,
                            accum_out=gates[:, slot : slot + 1],
                        )

                def emit_pair_tail(g1, gates, sp):
                    nc.vector.reduce_sum(
                        gates[:, 0 : 2 * JD], g1[:, 0 : 2 * JD, :],
                        axis=mybir.AxisListType.X,
                    )
                    for sl in range(2):
                        s = 2 * sp + sl
                        # two exps per slab: DVE-produced gate slots and
                        # ACT-produced ones are not contiguous.
                        nc.scalar.activation(
                            out=wall[:, s * JJ : s * JJ + JD],
                            in_=gates[:, sl * JD : (sl + 1) * JD],
                            func=mybir.ActivationFunctionType.Exp,
                            bias=b1b,
                            scale=1.0,
                        )
                        nc.scalar.activation(
                            out=wall[:, s * JJ + JD : (s + 1) * JJ],
                            in_=gates[:, 2 * JD + ND * sl : 2 * JD + ND * (sl + 1)],
                            func=mybir.ActivationFunctionType.Exp,
                            bias=b1b,
                            scale=1.0,
                        )
                        for t in range(JJ // 2):
                            a = slot_of(sl, 2 * t)
                            c = slot_of(sl, 2 * t + 1)
                            base = g1[:, a, :]
                            rhs = bass.AP(
                                tensor=base.tensor,
                                offset=base.offset,
                                ap=[list(base.ap[0]), [(c - a) * F, 2], [1, F]],
                            )
                            nc.tensor.matmul(
                                out=psum_row,
                                lhsT=wall[:, s * JJ + 2 * t : s * JJ + 2 * t + 2],
                                rhs=rhs,
                                start=(s == 0 and t == 0),
                                stop=(s == nslab - 1 and t == JJ // 2 - 1),
                            )

                for sp in range(nslab // 2):
                    g1 = g1p.tile([P, 2 * JJ, F], BF16, tag="g1")
                    gates = gatesp.tile([P, 2 * JJ], FP32, tag="gates")
                    # Stagger: the previous pair's reduce/exp/matmul tail is
                    # emitted BETWEEN this pair's two multiply blocks, so the
                    # DVE reduce phase (low SBUF pressure) overlaps one of
                    # the GpSimd multiply bursts instead of a DVE multiply.
                    emit_slab_front(g1, gates, sp, 0)
                    flush_pair_tail()
                    emit_slab_front(g1, gates, sp, 1)
                    pending[0] = (lambda g1=g1, gates=gates, sp=sp:
                                  emit_pair_tail(g1, gates, sp))
                flush_pair_tail()
                # Softmax denominator: one fused accumulate over the wall.
                dn = dnp.tile([P, 1], FP32, tag="dn")
                scr = scrp.tile([P, nslab * JJ], BF16, tag="scr3")
                nc.scalar.activation(
                    out=scr,
                    in_=wall,
                    func=mybir.ActivationFunctionType.Copy,
                    bias=0.0,
                    scale=1.0,
                    accum_out=dn,
                )
                yrow = outp.tile([2, 2 * F], FP32, tag="yrow", name=f"yrow_{b}")
                nc.scalar.copy(yrow, psum_row)
                outs.append((b, yrow, dn))
            # Output DMAs issued after every slab DMA is enqueued: the sync
            # queue never blocks the x stream on a graph's last matmul.
            for b, yrow, dn in outs:
                nc.sync.dma_start(out=y_d[b], in_=yrow)
                nc.sync.dma_start(out=dn_d[b], in_=dn)

    if fixup:
        # CoreSim chokes on the inserted NoOps; only needed for the HW compile.
        split_multiwait(nc)
    return nc


def run(x, W1, b1, trace: bool = False, tmpdir: str | None = None):
    """Shard over cores, execute, and return (y, BassKernelResults)."""
    import ml_dtypes

    x = np.ascontiguousarray(np.asarray(x, dtype=np.float32))
    W1 = np.ascontiguousarray(np.asarray(W1, dtype=np.float32))
    b1 = np.ascontiguousarray(np.asarray(b1, dtype=np.float32))
    assert x.shape == (BZ, N, F), x.shape

    nc = build()
    w1b16 = np.ascontiguousarray(np.tile(W1[:, 0].astype(ml_dtypes.bfloat16), JJ))
    in_maps = [
        {"x": np.ascontiguousarray(x[c * BZL : (c + 1) * BZL]), "W1b": w1b16,
         "b1": b1}
        for c in range(NCORES)
    ]
    res = run_bass_kernel_spmd(
        nc, in_maps, core_ids=list(range(NCORES)), trace=trace, tmpdir=tmpdir
    )
    y2 = np.concatenate([r["y_unnorm"] for r in res.results], axis=0)  # [32,2,512]
    y_un = y2[:, 0, 0:F] + y2[:, 1, F : 2 * F]                           # [32, 256]
    w1b = W1[:, 0].astype(ml_dtypes.bfloat16).astype(np.float64)
    y_un = y_un / w1b                   # PE consumed g1 = x*bf16(W1); undo
    dn = np.concatenate([r["denom"] for r in res.results], axis=0)   # [32,128,1]
    denom = dn.reshape(BZ, -1).astype(np.float64).sum(axis=1)
    y = (y_un / denom[:, None]).astype(np.float32)
    return y, res


def kernel(x, W1, b1):
    y, _ = run(x, W1, b1)
    return y
